# revision 1
# baseline (speedup 1.0000x reference)
"""Trainium2 Bass kernel for PVT-style spatial-reduction attention.

Shapes (hardcoded): x [2, 4096, 256], HEAD=8, dh=32, SR=2, R=8, H=W=64.
Sharding: core c = (batch b = c//4, query block j = c%4). Each core computes
q/attention/proj for its 1024 query rows and redundantly computes the small
conv+LN+KV path for its batch (no collectives; the kernel is ScalarE-exp
bound, so the redundant PE work hides).

Layouts: "transposed activations" — channels on partitions, tokens on the
free dim. Weights are pre-transposed/cast to bf16 on the host. Per-core x is
pre-rotated on host so each core's own query block is rows 0:1024 (softmax
over KV tokens is permutation invariant, and the 2x2/stride-2 conv commutes
with 16-image-row rotations).

PSUM budget (8 banks): scores 2x[128,1024] (4) + pv 2x[128,512] (2) +
conv/proj lane 1x[128,512] (1) + stats/kv lane 1x[128,512] (1).
"""
import sys

if "/opt/trn_rl_repo" not in sys.path:
    sys.path.insert(0, "/opt/trn_rl_repo")

import numpy as np
import ml_dtypes

BF16NP = ml_dtypes.bfloat16

HEAD, DH, C, N, B, M, R = 8, 32, 256, 4096, 2, 1024, 8
NB = N // 4          # query rows per core
SCALE = DH ** -0.5
NCORES = 8
MAGIC = 0x5F3759DF

_CACHE = {}


def _build_program():
    import concourse.bass as bass
    import concourse.tile as tile
    from concourse.bacc import Bacc
    from concourse import mybir, masks

    F32 = mybir.dt.float32
    BF16 = mybir.dt.bfloat16
    I32 = mybir.dt.int32
    AF = mybir.ActivationFunctionType
    ALU = mybir.AluOpType

    nc = Bacc()
    P = 128
    S = 2          # kv strips
    ST = 512       # kv tokens per strip

    def bcast(ap, nparts):
        # partition-stride-0 broadcast of a 1-D DRAM AP
        return bass.AP(tensor=ap.tensor, offset=ap.offset,
                       ap=[[0, nparts]] + [list(d) for d in ap.ap])

    # ---- DRAM parameters (host-prepped layouts) ----
    xT_d = nc.declare_dram_parameter("xT", [P, 2, N], BF16, isOutput=False)
    qwT_d = nc.declare_dram_parameter("qwT", [P, 2, C], BF16, isOutput=False)
    kvwT_d = nc.declare_dram_parameter("kvwT", [P, 2, 2 * C], BF16, isOutput=False)
    pwT_d = nc.declare_dram_parameter("pwT", [P, 2, C], BF16, isOutput=False)
    srwT_d = nc.declare_dram_parameter("srwT", [P, 2, 4, C], BF16, isOutput=False)
    aqT_d = nc.declare_dram_parameter("aqT", [P, 2, R], BF16, isOutput=False)
    avT_d = nc.declare_dram_parameter("avT", [P, 2, R], BF16, isOutput=False)
    bqT_d = nc.declare_dram_parameter("bqT", [R, 2, P], BF16, isOutput=False)
    bvT_d = nc.declare_dram_parameter("bvT", [R, 2, P], BF16, isOutput=False)
    qb_d = nc.declare_dram_parameter("qb", [P, 2], F32, isOutput=False)
    srb_d = nc.declare_dram_parameter("srb", [P, 2], F32, isOutput=False)
    wg1_d = nc.declare_dram_parameter("wg1", [1, 4, P], BF16, isOutput=False)
    avg1_d = nc.declare_dram_parameter("avg1", [1, R], BF16, isOutput=False)
    pb_d = nc.declare_dram_parameter("pb", [C], F32, isOutput=False)
    out_d = nc.declare_dram_parameter("out", [NB, C], F32, isOutput=True)

    with tile.TileContext(nc) as tc:
        with tc.tile_pool(name="wgt", bufs=1) as WGT, \
             tc.tile_pool(name="acts", bufs=1) as ACTS, \
             tc.tile_pool(name="strips", bufs=2) as STR, \
             tc.tile_pool(name="tmp", bufs=3) as TMP, \
             tc.tile_pool(name="atn", bufs=2) as ATN, \
             tc.tile_pool(name="pt", bufs=32) as PT, \
             tc.tile_pool(name="fin", bufs=2) as FIN, \
             tc.tile_pool(name="big", bufs=2, space="PSUM") as PSB, \
             tc.tile_pool(name="pv", bufs=2, space="PSUM") as PSV, \
             tc.tile_pool(name="cv", bufs=1, space="PSUM") as PSC, \
             tc.tile_pool(name="kvl", bufs=1, space="PSUM") as PSK, \
             tc.tile_pool(name="dscr", bufs=1, space="DRAM") as DSCR:

            # ---------- weights ----------
            def wload(name, shape, dt, src):
                t = WGT.tile(shape, dt, tag=name)
                nc.sync.dma_start(out=t[:], in_=src)
                return t

            qwT = wload("qwT", [P, 2, C], BF16, qwT_d[:])
            kvwT = wload("kvwT", [P, 2, 2 * C], BF16, kvwT_d[:])
            pwT = wload("pwT", [P, 2, C], BF16, pwT_d[:])
            srwT = wload("srwT", [P, 2, 4, C], BF16, srwT_d[:])
            aqT = wload("aqT", [P, 2, R], BF16, aqT_d[:])
            avT = wload("avT", [P, 2, R], BF16, avT_d[:])
            bqT = wload("bqT", [R, 2, P], BF16, bqT_d[:])
            bvT = wload("bvT", [R, 2, P], BF16, bvT_d[:])
            qb = wload("qb", [P, 2], F32, qb_d[:])
            srb = wload("srb", [P, 2], F32, srb_d[:])
            wg1t = wload("wg1", [1, 4, P], BF16, wg1_d[:])
            avg1t = wload("avg1", [1, R], BF16, avg1_d[:])
            pbB = wload("pbB", [P, C], F32, bcast(pb_d.ap(), P))
            ones1 = WGT.tile([P, 1], BF16, tag="ones1")
            nc.gpsimd.memset(ones1[:], 1.0 / C)
            ident = WGT.tile([P, P], BF16, tag="ident")
            masks.make_identity(nc, ident[:])

            # persistent activations
            qT = ACTS.tile([P, 2, NB], BF16, tag="qT")
            outT = ACTS.tile([P, 2, NB], BF16, tag="outT")
            tq = ACTS.tile([R, NB], BF16, tag="tq")

            xTs, kts, vsb, ans, ascl = [], [], [], [], []

            # ---------- per-strip setup + q path ----------
            for s in range(S):
                xs_t = ACTS.tile([P, 2, 2048], BF16, tag=f"xT{s}")
                nc.gpsimd.dma_start(out=xs_t[:], in_=xT_d[:, :, s * 2048:(s + 1) * 2048])
                xTs.append(xs_t)

                # conv (2x2 stride-2 as 8 accumulated matmuls per out-chunk)
                xs_s = STR.tile([P, 2, ST], F32, tag="xs")
                for oc in range(2):
                    cps = PSC.tile([P, ST], F32, tag="cv")
                    first = True
                    for cc in range(2):
                        xv = xs_t[:, cc, :].rearrange(
                            "p (i a j b) -> p i a j b", i=16, a=2, j=32, b=2)
                        for di in range(2):
                            for dj in range(2):
                                nc.tensor.matmul(
                                    cps[:], srwT[:, cc, di * 2 + dj,
                                                 oc * P:(oc + 1) * P],
                                    xv[:, :, di, :, dj],
                                    start=first,
                                    stop=(cc == 1 and di == 1 and dj == 1))
                                first = False
                    nc.vector.tensor_scalar_add(
                        out=xs_s[:, oc, :], in0=cps[:], scalar1=srb[:, oc:oc + 1])

                # LN stats via (1/C)-ones matmul channel sums -> mean/E[x^2]
                xsb_s = STR.tile([P, 2, ST], BF16, tag="xsb")
                nc.gpsimd.tensor_copy(out=xsb_s[:], in_=xs_s[:])
                sq_s = STR.tile([P, 2, ST], BF16, tag="sq")
                nc.vector.tensor_mul(out=sq_s[:], in0=xsb_s[:], in1=xsb_s[:])
                sxp = PSK.tile([1, ST], F32, tag="kvl")
                nc.tensor.matmul(sxp[:], ones1[:], xsb_s[:, 0, :], start=True, stop=False)
                nc.tensor.matmul(sxp[:], ones1[:], xsb_s[:, 1, :], start=False, stop=True)
                negmu = TMP.tile([1, ST], BF16, tag="negmu")
                nc.vector.tensor_scalar_mul(out=negmu[:], in0=sxp[:], scalar1=-1.0)
                sxxp = PSK.tile([1, ST], F32, tag="kvl")
                nc.tensor.matmul(sxxp[:], ones1[:], sq_s[:, 0, :], start=True, stop=False)
                nc.tensor.matmul(sxxp[:], ones1[:], sq_s[:, 1, :], start=False, stop=True)
                ex2_sb = TMP.tile([1, ST], F32, tag="ex2sb")
                nc.vector.tensor_copy(out=ex2_sb[:], in_=sxxp[:])
                # chunk-major repack [1, 512] -> [128, 4]  (t = g*128 + p)
                # via DRAM bounce (SBUF source APs can't express the permute)
                nm_d = DSCR.tile([ST], BF16, tag=f"nm{s}")
                nc.sync.dma_start(out=nm_d[:], in_=negmu[:])
                ex_d = DSCR.tile([ST], F32, tag=f"ex{s}")
                nc.sync.dma_start(out=ex_d[:], in_=ex2_sb[:])
                mur = TMP.tile([P, 4], BF16, tag="mur")
                nc.sync.dma_start(out=mur[:],
                                  in_=nm_d[:].rearrange("(g p) -> p g", p=P))
                ex2r = TMP.tile([P, 4], F32, tag="ex2r")
                nc.sync.dma_start(out=ex2r[:],
                                  in_=ex_d[:].rearrange("(g p) -> p g", p=P))
                # rstd via quake rsqrt (1 newton); an = rstd, ascl = SCALE*rstd
                nmu2 = TMP.tile([P, 4], F32, tag="nmu2")
                nc.vector.scalar_tensor_tensor(out=nmu2[:], in0=mur[:], scalar=-1.0,
                                               in1=mur[:], op0=ALU.mult, op1=ALU.mult)
                ve = TMP.tile([P, 4], F32, tag="ve")
                nc.vector.scalar_tensor_tensor(out=ve[:], in0=nmu2[:], scalar=1e-5,
                                               in1=ex2r[:], op0=ALU.add, op1=ALU.add)
                hsh = TMP.tile([P, 4], I32, tag="hsh")
                nc.vector.tensor_scalar(out=hsh[:], in0=ve[:].bitcast(I32), scalar1=1,
                                        scalar2=None, op0=ALU.logical_shift_right)
                nc.vector.tensor_scalar(out=hsh[:], in0=hsh[:], scalar1=-1,
                                        scalar2=MAGIC, op0=ALU.mult, op1=ALU.add)
                y0 = hsh[:].bitcast(F32)
                nt = TMP.tile([P, 4], F32, tag="nt")
                nc.vector.tensor_mul(out=nt[:], in0=y0, in1=y0)
                nc.vector.scalar_tensor_tensor(out=nt[:], in0=nt[:], scalar=-0.5,
                                               in1=ve[:], op0=ALU.mult, op1=ALU.mult)
                nc.vector.tensor_scalar_add(out=nt[:], in0=nt[:], scalar1=1.5)
                an_s = STR.tile([P, 4], F32, tag="an")
                nc.vector.tensor_mul(out=an_s[:], in0=y0, in1=nt[:])
                ascl_s = STR.tile([P, 4], F32, tag="ascl")
                nc.vector.tensor_scalar_mul(out=ascl_s[:], in0=an_s[:], scalar1=SCALE)
                ans.append(an_s)
                ascl.append(ascl_s)

                # shared lora for k and v: t2raw = Avg @ xs_raw - mu*avg1
                t2p = PSK.tile([R, ST], F32, tag="kvl")
                nc.tensor.matmul(t2p[:], avT[:, 0, :], xsb_s[:, 0, :], start=True, stop=False)
                nc.tensor.matmul(t2p[:], avT[:, 1, :], xsb_s[:, 1, :], start=False, stop=False)
                nc.tensor.matmul(t2p[:], avg1t[:], negmu[:], start=False, stop=True)
                t2 = TMP.tile([R, ST], BF16, tag="t2")
                nc.vector.tensor_copy(out=t2[:], in_=t2p[:])

                kts_s = STR.tile([P, 2, ST], BF16, tag="kts")
                vtmp_s = STR.tile([P, 2, ST], BF16, tag="vtmp")
                for kvoc in range(4):
                    kps = PSK.tile([P, ST], F32, tag="kvl")
                    nc.tensor.matmul(kps[:], kvwT[:, 0, kvoc * P:(kvoc + 1) * P],
                                     xsb_s[:, 0, :], start=True, stop=False)
                    nc.tensor.matmul(kps[:], kvwT[:, 1, kvoc * P:(kvoc + 1) * P],
                                     xsb_s[:, 1, :], start=False, stop=False)
                    nc.tensor.matmul(kps[:], wg1t[:, kvoc, :], negmu[:],
                                     start=False, stop=False)
                    nc.tensor.matmul(kps[:], bvT[:, kvoc % 2, :], t2[:],
                                     start=False, stop=True)
                    dst = kts_s[:, kvoc, :] if kvoc < 2 else vtmp_s[:, kvoc - 2, :]
                    nc.vector.tensor_copy(out=dst, in_=kps[:])
                kts.append(kts_s)

                # v transpose to [m, c] (PE transpose) + ones column
                vsb_s = STR.tile([P, 4, HEAD, DH + 1], BF16, tag="vsb")
                for vc in range(2):
                    for u4 in range(4):
                        vtp = PSK.tile([P, P], BF16, tag="kvl")
                        nc.tensor.transpose(vtp[:],
                                            vtmp_s[:, vc, u4 * P:(u4 + 1) * P],
                                            ident[:])
                        nc.vector.tensor_scalar_mul(
                            out=vsb_s[:, u4, vc * 4:(vc + 1) * 4, 0:DH],
                            in0=vtp[:].rearrange("p (h d) -> p h d", d=DH),
                            scalar1=an_s[:, u4:u4 + 1])
                nc.gpsimd.memset(vsb_s[:, :, :, DH:DH + 1], 1.0)
                vsb.append(vsb_s)

                if s == 0:
                    # q path (only needs x rows 0:1024 = first half of strip 0)
                    tqp = PSB.tile([R, NB], F32, tag="big")
                    for nh in range(2):
                        sl = slice(nh * 512, (nh + 1) * 512)
                        nc.tensor.matmul(tqp[:, sl], aqT[:, 0, :], xs_t[:, 0, sl],
                                         start=True, stop=False)
                        nc.tensor.matmul(tqp[:, sl], aqT[:, 1, :], xs_t[:, 1, sl],
                                         start=False, stop=True)
                    nc.vector.tensor_copy(out=tq[:], in_=tqp[:])
                    for oc in range(2):
                        qps = PSB.tile([P, NB], F32, tag="big")
                        for nh in range(2):
                            sl = slice(nh * 512, (nh + 1) * 512)
                            nc.tensor.matmul(qps[:, sl],
                                             qwT[:, 0, oc * P:(oc + 1) * P],
                                             xs_t[:, 0, sl], start=True, stop=False)
                            nc.tensor.matmul(qps[:, sl],
                                             qwT[:, 1, oc * P:(oc + 1) * P],
                                             xs_t[:, 1, sl], start=False, stop=False)
                            nc.tensor.matmul(qps[:, sl], bqT[:, oc, :], tq[:, sl],
                                             start=False, stop=True)
                        nc.vector.tensor_scalar_add(
                            out=qT[:, oc, :], in0=qps[:], scalar1=qb[:, oc:oc + 1])


            # ---------- attention: 4 head pairs, software-pipelined ----------
            def emit_scores(g, mc, pts):
                ch, r0 = g // 2, 64 * (g % 2)
                s, ml = mc // 4, mc % 4
                for h01 in range(2):
                    rr = r0 + 32 * h01
                    stile = PSB.tile([P, NB], F32, tag="big")
                    lhsT = kts[s][rr:rr + 32, ch, ml * P:(ml + 1) * P]
                    for nh in range(2):
                        sl = slice(nh * 512, (nh + 1) * 512)
                        nc.tensor.matmul(stile[:, sl], lhsT,
                                         qT[rr:rr + 32, ch, sl],
                                         start=True, stop=True,
                                         tile_position=(rr, 0))
                    pt_t = PT.tile([P, NB], BF16, tag="pt")
                    nc.scalar.activation(out=pt_t[:], in_=stile[:],
                                         func=AF.Exp,
                                         scale=ascl[s][:, ml:ml + 1])
                    pts[(h01, mc)] = pt_t

            def pv_mm(g, nh, pvp, pts, mc):
                sl = slice(nh * 512, (nh + 1) * 512)
                s, ml = mc // 4, mc % 4
                for h01 in range(2):
                    h = 2 * g + h01
                    nc.tensor.matmul(
                        pvp[64 * h01:64 * h01 + DH + 1, :],
                        vsb[s][:, ml, h, :], pts[(h01, mc)][:, sl],
                        start=(mc == 0), stop=(mc == 7),
                        tile_position=(0, 64 * h01))

            def pv_tail(g, nh, pvp, rec, fac, tmpo, rec_s):
                ch, r0 = g // 2, 64 * (g % 2)
                sl = slice(nh * 512, (nh + 1) * 512)
                # softmax denominators -> factors (DRAM-bounce broadcast)
                nc.vector.reciprocal(out=rec[0:1, sl], in_=pvp[DH:DH + 1, :])
                nc.vector.reciprocal(out=rec[32:33, sl], in_=pvp[64 + DH:64 + DH + 1, :])
                nc.sync.dma_start(out=rec_s[0, sl], in_=rec[0:1, sl])
                nc.sync.dma_start(out=rec_s[1, sl], in_=rec[32:33, sl])
                for h01 in range(2):
                    nc.sync.dma_start(out=fac[64 * h01:64 * h01 + DH, sl],
                                      in_=bcast(rec_s[h01, sl], DH))
                for h01 in range(2):
                    nc.vector.tensor_mul(out=tmpo[64 * h01:64 * h01 + DH, sl],
                                         in0=pvp[64 * h01:64 * h01 + DH, :],
                                         in1=fac[64 * h01:64 * h01 + DH, sl])
                    nc.scalar.dma_start(
                        out=outT[r0 + 32 * h01:r0 + 32 * h01 + 32, ch, sl],
                        in_=tmpo[64 * h01:64 * h01 + DH, sl])

            for g in range(4):
                pts = {}
                rec = ATN.tile([33, NB], F32, tag="rec")
                fac = ATN.tile([P, NB], F32, tag="fac")
                tmpo = ATN.tile([P, NB], BF16, tag="tmpo")
                rec_s = DSCR.tile([2, NB], F32, tag=f"rec{g}")
                pvp0 = PSV.tile([P, 512], F32, tag="pv")
                pvp1 = PSV.tile([P, 512], F32, tag="pv")
                for mc in range(8):
                    emit_scores(g, mc, pts)
                    pv_mm(g, 0, pvp0, pts, mc)
                    pv_mm(g, 1, pvp1, pts, mc)
                pv_tail(g, 0, pvp0, rec, fac, tmpo, rec_s)
                pv_tail(g, 1, pvp1, rec, fac, tmpo, rec_s)

            # ---------- output projection ----------
            for t8 in range(8):
                pp = PSC.tile([P, C], F32, tag="cv")
                nc.tensor.matmul(pp[:], outT[:, 0, t8 * P:(t8 + 1) * P],
                                 pwT[:, 0, :], start=True, stop=False)
                nc.tensor.matmul(pp[:], outT[:, 1, t8 * P:(t8 + 1) * P],
                                 pwT[:, 1, :], start=False, stop=True)
                fin = FIN.tile([P, C], F32, tag="fin")
                nc.vector.tensor_add(out=fin[:], in0=pp[:], in1=pbB[:])
                nc.scalar.dma_start(out=out_d[t8 * P:(t8 + 1) * P, :], in_=fin[:])

    nc.finalize()
    return nc


def _prep_shared(q_w, q_b, kv_w, kv_b, proj_w, proj_b, a_q, b_q, a_v, b_v,
                 sr_w, sr_b, ln_g, ln_b):
    f32 = np.float32

    def chunkT(w):  # [in, out] -> [128, n_in_chunks, out]
        wt = np.ascontiguousarray(np.asarray(w, f32).T)
        ic, oc = wt.shape
        return np.ascontiguousarray(
            wt.reshape(ic // 128, 128, oc).transpose(1, 0, 2)).astype(BF16NP)

    def pcols(v):  # [n*128] -> [128, n]
        v = np.asarray(v, f32)
        return np.ascontiguousarray(v.reshape(-1, 128).T)

    kv_w = np.asarray(kv_w, f32)
    a_v = np.asarray(a_v, f32)
    b_v = np.asarray(b_v, f32)
    g = np.asarray(ln_g, f32)
    bb = np.asarray(ln_b, f32)
    proj_w = np.asarray(proj_w, f32)
    # fold LayerNorm gamma into kv/a_v weights; mean via rank-1 correction;
    # k-side constants dropped (softmax shift invariance), v-side constants
    # folded into the projection bias.
    Wg = kv_w * g[None, :]
    wg1 = Wg.sum(1)
    Avg = a_v * g[None, :]
    avg1 = Avg.sum(1)
    wbt = kv_w @ bb + np.asarray(kv_b, f32)
    dconst = b_v @ (a_v @ bb)
    wv_const = wbt[C:] + dconst
    pb_eff = np.asarray(proj_b, f32) + proj_w @ wv_const

    srwT = np.asarray(sr_w, f32).transpose(1, 2, 3, 0).reshape(2, 128, 4, C)
    srwT = np.ascontiguousarray(srwT.transpose(1, 0, 2, 3)).astype(BF16NP)
    bqT = np.ascontiguousarray(np.asarray(b_q, f32).T.reshape(R, 2, 128)).astype(BF16NP)
    bvT = np.ascontiguousarray(b_v.T.reshape(R, 2, 128)).astype(BF16NP)
    return dict(
        qwT=chunkT(q_w), kvwT=chunkT(Wg), pwT=chunkT(proj_w),
        srwT=srwT, aqT=chunkT(a_q), avT=chunkT(Avg), bqT=bqT, bvT=bvT,
        qb=pcols(q_b), srb=pcols(sr_b),
        wg1=np.ascontiguousarray(wg1.reshape(1, 4, 128)).astype(BF16NP),
        avg1=np.ascontiguousarray(avg1.reshape(1, R)).astype(BF16NP),
        pb=pb_eff,
    )


def kernel(x, q_w, q_b, kv_w, kv_b, proj_w, proj_b, a_q, b_q, a_v, b_v,
           sr_w, sr_b, ln_g, ln_b, H, W):
    from concourse.bass_utils import run_bass_kernel_spmd

    x = np.asarray(x, np.float32)
    assert x.shape == (B, N, C) and int(H) == 64 and int(W) == 64

    if "nc" not in _CACHE:
        _CACHE["nc"] = _build_program()
    nc = _CACHE["nc"]

    shared = _prep_shared(q_w, q_b, kv_w, kv_b, proj_w, proj_b, a_q, b_q,
                          a_v, b_v, sr_w, sr_b, ln_g, ln_b)
    in_maps = []
    for c in range(NCORES):
        b, j = c // 4, c % 4
        xb = np.roll(x[b], -NB * j, axis=0)          # own block at rows 0:1024
        xT = np.ascontiguousarray(xb.T.astype(BF16NP))  # [256, 4096]
        xT = np.ascontiguousarray(
            xT.reshape(2, 128, N).transpose(1, 0, 2))   # [128, 2, 4096]
        in_maps.append(dict(shared, xT=xT))

    res = run_bass_kernel_spmd(nc, in_maps, list(range(NCORES)))
    out = np.empty((B, N, C), np.float32)
    for c in range(NCORES):
        b, j = c // 4, c % 4
        out[b, NB * j:NB * (j + 1)] = res.results[c]["out"]
    return out

